# revision 2
# baseline (speedup 1.0000x reference)
"""Trainium2 Bass kernel for nn_MetadataEncoder (embedding_lookup).

Math: out = lrelu(concat(emb, cont) @ W.T + b), emb = 21 table lookups.
Since the MLP is linear, fold W into the tables on the host:
    P_t = tables[t] @ W[:, 16t:16t+16].T           -> [1000, 32] each
and fold the continuous branch + bias into a quantized 22nd table:
    C[q] = lrelu(x_q @ W_c.T + b_c) @ W[:, 336:344].T + b,  x_q = (q+.5)/Q
Then per row r: out[r] = lrelu( sum_t P_t[x_cat[r,t]] + C[quant(x_cont[r])] ).

Device work per core (data-parallel over 8 cores, batch-sharded):
  - dma_gather (SWDGE, 256B descriptors) from an HBM table of 64-f32 rows
    (first 32 columns = payload), 22 gathers per row block
  - DVE strided adds to accumulate the 22 slabs (f32)
  - ScalarE Lrelu epilogue, DMA out.
Output is produced in [128, nblk, 32] row-wrapped layout (row r at
partition r%128), unwrapped on the host.
"""
import numpy as np

import concourse.bacc as bacc
import concourse.mybir as mybir
import concourse.tile as tile
from concourse.bass_utils import run_bass_kernel_spmd

NUM_TABLES = 21
NUM_CATS = 1000
EMB_DIM = 16
B = 500000
OUT_DIM = 32
NEG_SLOPE = 0.01
QCONT = 8192                      # cont-branch quantization levels
N_CORES = 8
SHARD = B // N_CORES              # 62500
SHARD_PAD = 62592                 # = 489 * 128, % 16 == 0
TAB_ROWS = NUM_TABLES * NUM_CATS + QCONT          # 23048
TAB_ROWS_PAD = 29696
ELEM = 64                         # gathered f32 per idx (256B); first 32 used
CB_MAIN = 4096                    # rows per block
NQ = 4

_cache = {}


def _blocks():
    blks = []
    r = 0
    while r < SHARD_PAD:
        cb = min(CB_MAIN, SHARD_PAD - r)
        blks.append((r, cb))
        r += cb
    return blks


def _build(reps=1, nt=None, nq=None, reduce_on=True):
    nc = bacc.Bacc("TRN2", target_bir_lowering=False,
                   num_swdge_queues=(NQ if nq is None else nq),
                   dynamic_dma_scratch_size=131072)
    f32, i16 = mybir.dt.float32, mybir.dt.int16
    blks = _blocks()
    idx_cols_total = sum((cb // 16) * (NUM_TABLES + 1) for _, cb in blks)

    tab_d = nc.dram_tensor("tab", [TAB_ROWS_PAD, ELEM], f32, kind="ExternalInput")
    idx_d = nc.dram_tensor("idx", [16, idx_cols_total], i16, kind="ExternalInput")
    out_d = nc.dram_tensor("out", [128, (SHARD_PAD // 128) * OUT_DIM], f32,
                           kind="ExternalOutput")

    NT = (NUM_TABLES + 1) if nt is None else nt
    with tile.TileContext(nc) as tc:
        with tc.tile_pool(name="idxp", bufs=2) as idxp, \
             tc.tile_pool(name="slabp", bufs=4) as slabp, \
             tc.tile_pool(name="accp", bufs=2) as accp, \
             tc.tile_pool(name="outp", bufs=2) as outp:
          for rep in range(reps):
            icol = 0
            for bi, (r0, cb) in enumerate(blks):
                ic = (cb // 16) * NT
                nrow = cb // 128
                # load compact idx [16, ic] and replicate to all 8 groups
                idx_sb = idxp.tile([128, ic], i16, tag="idx")
                nc.sync.dma_start(idx_sb[0:16, :], idx_d[:, icol:icol + ic])
                for g in range(1, 8):
                    nc.sync.dma_start(idx_sb[16 * g:16 * g + 16, :], idx_sb[0:16, :])
                icol += ic

                acc = accp.tile([128, nrow * OUT_DIM], f32, tag="acc")
                for t in range(NT):
                    slab = slabp.tile([128, nrow * ELEM], f32, tag="slab")
                    nc.gpsimd.dma_gather(
                        slab[:].rearrange("p (n e) -> p n e", e=ELEM),
                        tab_d[:],
                        idx_sb[:, t * (cb // 16):(t + 1) * (cb // 16)],
                        num_idxs=cb,
                        num_idxs_reg=cb,
                        elem_size=ELEM,
                        queue_num=(bi * NT + t) % (NQ if nq is None else nq),
                        single_packet=False,
                    )
                    # strided view: slab[p, n, 0:32]
                    if reduce_on:
                        sv = slab[:].rearrange("p (n e) -> p n e", e=ELEM)[:, :, 0:OUT_DIM]
                        av = acc[:].rearrange("p (n o) -> p n o", o=OUT_DIM)
                        if t == 0:
                            nc.vector.tensor_copy(av, sv)
                        else:
                            nc.vector.tensor_tensor(av, av, sv, mybir.AluOpType.add)
                if not reduce_on:
                    nc.vector.tensor_copy(
                        acc[:], slab[:].rearrange("p (n e) -> p n e", e=ELEM)[:, :, 0:OUT_DIM])
                ob = outp.tile([128, nrow * OUT_DIM], f32, tag="ob")
                nc.vector.tensor_scalar(ob[:], acc[:], NEG_SLOPE, None,
                                        mybir.AluOpType.mult)
                nc.vector.tensor_tensor(ob[:], ob[:], acc[:],
                                        mybir.AluOpType.max)
                nc.sync.dma_start(
                    out_d[:, r0 // 128 * OUT_DIM:(r0 // 128 + nrow) * OUT_DIM], ob[:])
    nc.compile()
    return nc


def _prep_tables(tables, W_c, b_c, W, b):
    """Host fold: [TAB_ROWS_PAD, 64] f32 table."""
    tab = np.zeros((TAB_ROWS_PAD, ELEM), np.float32)
    td = tables.astype(np.float64)
    Wd = W.astype(np.float64)
    for t in range(NUM_TABLES):
        P = td[t] @ Wd[:, EMB_DIM * t:EMB_DIM * (t + 1)].T       # [1000, 32]
        tab[NUM_CATS * t:NUM_CATS * (t + 1), :OUT_DIM] = P.astype(np.float32)
    xq = (np.arange(QCONT, dtype=np.float64) + 0.5) / QCONT      # [Q]
    v = xq[:, None] * W_c.astype(np.float64)[None, :, 0] + b_c.astype(np.float64)[None, :]
    v = np.where(v >= 0, v, NEG_SLOPE * v)                       # [Q, 8]
    C = v @ Wd[:, NUM_TABLES * EMB_DIM:].T + b.astype(np.float64)[None, :]
    tab[NUM_TABLES * NUM_CATS:NUM_TABLES * NUM_CATS + QCONT, :OUT_DIM] = C.astype(np.float32)
    return tab


def _prep_idx(x_cat_shard, x_cont_shard):
    """[16, idx_cols_total] int16 for one core, 16-wrapped, block/table-major."""
    n = x_cat_shard.shape[0]
    gi = np.zeros((SHARD_PAD, NUM_TABLES + 1), np.int16)
    base = (np.arange(NUM_TABLES, dtype=np.int32) * NUM_CATS)[None, :]
    gi[:n, :NUM_TABLES] = (x_cat_shard.astype(np.int32) + base).astype(np.int16)
    q = np.clip((x_cont_shard[:, 0] * QCONT).astype(np.int32), 0, QCONT - 1)
    gi[:n, NUM_TABLES] = (NUM_TABLES * NUM_CATS + q).astype(np.int16)
    cols = []
    for r0, cb in _blocks():
        blk = gi[r0:r0 + cb]                       # [cb, NT]
        # per table t: idx j at (j % 16, j // 16)
        w = blk.T.reshape(NUM_TABLES + 1, cb // 16, 16)   # [NT, cb/16, 16]
        cols.append(np.ascontiguousarray(w.transpose(0, 2, 1))
                    .reshape(NUM_TABLES + 1, 16, cb // 16)
                    .transpose(1, 0, 2).reshape(16, -1))
    return np.concatenate(cols, axis=1)


def core0_inputs(inputs):
    """Core-0 input map for timing builds."""
    tab = _prep_tables(inputs["tables"], inputs["W_c"], inputs["b_c"],
                       inputs["W"], inputs["b"])
    idx = _prep_idx(inputs["x_cat"][:SHARD], inputs["x_cont"][:SHARD])
    return {"tab": tab, "idx": idx}


def kernel(x_cat, x_cont, tables, W_c, b_c, W, b):
    if "nc" not in _cache:
        _cache["nc"] = _build()
    nc = _cache["nc"]
    tab = _prep_tables(tables, W_c, b_c, W, b)
    in_maps = []
    for c in range(N_CORES):
        sl = slice(c * SHARD, (c + 1) * SHARD)
        in_maps.append({
            "tab": tab,
            "idx": _prep_idx(np.asarray(x_cat)[sl], np.asarray(x_cont)[sl]),
        })
    res = run_bass_kernel_spmd(nc, in_maps, core_ids=list(range(N_CORES)))
    outs = []
    for c in range(N_CORES):
        o = res.results[c]["out"].reshape(128, SHARD_PAD // 128, OUT_DIM)
        o = o.transpose(1, 0, 2).reshape(SHARD_PAD, OUT_DIM)[:SHARD]
        outs.append(o)
    return np.ascontiguousarray(np.concatenate(outs, axis=0))



# revision 3
# speedup vs baseline: 1.8880x; 1.8880x over previous
"""Trainium2 Bass kernel for nn_MetadataEncoder (embedding_lookup).

Math: out = lrelu(concat(emb, cont) @ W.T + b), emb = 21 table lookups.
Since the MLP is linear, fold W into the tables on the host:
    P_t = tables[t] @ W[:, 16t:16t+16].T           -> [1000, 32] each
and fold the continuous branch + bias into a quantized 22nd table:
    C[q] = lrelu(x_q @ W_c.T + b_c) @ W[:, 336:344].T + b,  x_q = (q+.5)/Q
Then per row r: out[r] = lrelu( sum_t P_t[x_cat[r,t]] + C[quant(x_cont[r])] ).

Device work per core (data-parallel over 8 cores, batch-sharded):
  - dma_gather (SWDGE, 256B descriptors) from an HBM table of 64-f32 rows
    (first 32 columns = payload), 22 gathers per row block
  - DVE strided adds to accumulate the 22 slabs (f32)
  - ScalarE Lrelu epilogue, DMA out.
Output is produced in [128, nblk, 32] row-wrapped layout (row r at
partition r%128), unwrapped on the host.
"""
import numpy as np

import concourse.bacc as bacc
import concourse.mybir as mybir
import concourse.tile as tile
from concourse.bass_utils import run_bass_kernel_spmd

NUM_TABLES = 21
NUM_CATS = 1000
EMB_DIM = 16
B = 500000
OUT_DIM = 32
NEG_SLOPE = 0.01
QCONT = 8192                      # cont-branch quantization levels
SINGLE_PACKET = False
N_CORES = 8
SHARD = B // N_CORES              # 62500
SHARD_PAD = 62592                 # = 489 * 128, % 16 == 0
TAB_ROWS = NUM_TABLES * NUM_CATS + QCONT          # 23048
TAB_ROWS_PAD = 29696
ELEM = 64                         # gathered f32 per idx (256B); first 32 used
CB_MAIN = 4096                    # rows per block
NQ = 4

_cache = {}


def _blocks():
    blks = []
    r = 0
    while r < SHARD_PAD:
        cb = min(CB_MAIN, SHARD_PAD - r)
        blks.append((r, cb))
        r += cb
    return blks


def _build(reps=1, nt=None, nq=None, reduce_on=True):
    nc = bacc.Bacc("TRN2", target_bir_lowering=False,
                   num_swdge_queues=(NQ if nq is None else nq),
                   dynamic_dma_scratch_size=131072)
    f32, i16 = mybir.dt.float32, mybir.dt.int16
    blks = _blocks()
    idx_cols_total = sum((cb // 16) * (NUM_TABLES + 1) for _, cb in blks)

    tab_d = nc.dram_tensor("tab", [TAB_ROWS_PAD, ELEM], f32, kind="ExternalInput")
    idx_d = nc.dram_tensor("idx", [16, idx_cols_total], i16, kind="ExternalInput")
    out_d = nc.dram_tensor("out", [128, (SHARD_PAD // 128) * OUT_DIM], f32,
                           kind="ExternalOutput")

    NT = (NUM_TABLES + 1) if nt is None else nt
    with tile.TileContext(nc) as tc:
        with tc.tile_pool(name="idxp", bufs=2) as idxp, \
             tc.tile_pool(name="slabp", bufs=4) as slabp, \
             tc.tile_pool(name="accp", bufs=2) as accp, \
             tc.tile_pool(name="outp", bufs=2) as outp:
          for rep in range(reps):
            icol = 0
            for bi, (r0, cb) in enumerate(blks):
                ic = (cb // 16) * NT
                nrow = cb // 128
                # load compact idx [16, ic] and replicate to all 8 groups
                idx_sb = idxp.tile([128, ic], i16, tag="idx")
                nc.sync.dma_start(idx_sb[0:16, :], idx_d[:, icol:icol + ic])
                for g in range(1, 8):
                    nc.sync.dma_start(idx_sb[16 * g:16 * g + 16, :], idx_sb[0:16, :])
                icol += ic

                acc = accp.tile([128, nrow * OUT_DIM], f32, tag="acc")
                for t in range(NT):
                    slab = slabp.tile([128, nrow * ELEM], f32, tag="slab")
                    nc.gpsimd.dma_gather(
                        slab[:].rearrange("p (n e) -> p n e", e=ELEM),
                        tab_d[:],
                        idx_sb[:, t * (cb // 16):(t + 1) * (cb // 16)],
                        num_idxs=cb,
                        num_idxs_reg=cb,
                        elem_size=ELEM,
                        queue_num=(bi * NT + t) % (NQ if nq is None else nq),
                        single_packet=SINGLE_PACKET,
                    )
                    # strided view: slab[p, n, 0:32]
                    if reduce_on:
                        sv = slab[:].rearrange("p (n e) -> p n e", e=ELEM)[:, :, 0:OUT_DIM]
                        av = acc[:].rearrange("p (n o) -> p n o", o=OUT_DIM)
                        if t == 0:
                            nc.vector.tensor_copy(av, sv)
                        else:
                            nc.vector.tensor_tensor(av, av, sv, mybir.AluOpType.add)
                if not reduce_on:
                    nc.vector.tensor_copy(
                        acc[:], slab[:].rearrange("p (n e) -> p n e", e=ELEM)[:, :, 0:OUT_DIM])
                ob = outp.tile([128, nrow * OUT_DIM], f32, tag="ob")
                nc.vector.tensor_scalar(ob[:], acc[:], NEG_SLOPE, None,
                                        mybir.AluOpType.mult)
                nc.vector.tensor_tensor(ob[:], ob[:], acc[:],
                                        mybir.AluOpType.max)
                nc.sync.dma_start(
                    out_d[:, r0 // 128 * OUT_DIM:(r0 // 128 + nrow) * OUT_DIM], ob[:])
    nc.compile()
    return nc


def _prep_tables(tables, W_c, b_c, W, b):
    """Host fold: [TAB_ROWS_PAD, 64] f32 table."""
    tab = np.zeros((TAB_ROWS_PAD, ELEM), np.float32)
    td = tables.astype(np.float64)
    Wd = W.astype(np.float64)
    for t in range(NUM_TABLES):
        P = td[t] @ Wd[:, EMB_DIM * t:EMB_DIM * (t + 1)].T       # [1000, 32]
        tab[NUM_CATS * t:NUM_CATS * (t + 1), :OUT_DIM] = P.astype(np.float32)
    xq = (np.arange(QCONT, dtype=np.float64) + 0.5) / QCONT      # [Q]
    v = xq[:, None] * W_c.astype(np.float64)[None, :, 0] + b_c.astype(np.float64)[None, :]
    v = np.where(v >= 0, v, NEG_SLOPE * v)                       # [Q, 8]
    C = v @ Wd[:, NUM_TABLES * EMB_DIM:].T + b.astype(np.float64)[None, :]
    tab[NUM_TABLES * NUM_CATS:NUM_TABLES * NUM_CATS + QCONT, :OUT_DIM] = C.astype(np.float32)
    return tab


def _prep_idx(x_cat_shard, x_cont_shard):
    """[16, idx_cols_total] int16 for one core, 16-wrapped, block/table-major."""
    n = x_cat_shard.shape[0]
    gi = np.zeros((SHARD_PAD, NUM_TABLES + 1), np.int16)
    base = (np.arange(NUM_TABLES, dtype=np.int32) * NUM_CATS)[None, :]
    gi[:n, :NUM_TABLES] = (x_cat_shard.astype(np.int32) + base).astype(np.int16)
    q = np.clip((x_cont_shard[:, 0] * QCONT).astype(np.int32), 0, QCONT - 1)
    gi[:n, NUM_TABLES] = (NUM_TABLES * NUM_CATS + q).astype(np.int16)
    cols = []
    for r0, cb in _blocks():
        blk = gi[r0:r0 + cb]                       # [cb, NT]
        # per table t: idx j at (j % 16, j // 16)
        w = blk.T.reshape(NUM_TABLES + 1, cb // 16, 16)   # [NT, cb/16, 16]
        cols.append(np.ascontiguousarray(w.transpose(0, 2, 1))
                    .reshape(NUM_TABLES + 1, 16, cb // 16)
                    .transpose(1, 0, 2).reshape(16, -1))
    return np.concatenate(cols, axis=1)


def core0_inputs(inputs):
    """Core-0 input map for timing builds."""
    tab = _prep_tables(inputs["tables"], inputs["W_c"], inputs["b_c"],
                       inputs["W"], inputs["b"])
    idx = _prep_idx(inputs["x_cat"][:SHARD], inputs["x_cont"][:SHARD])
    return {"tab": tab, "idx": idx}


def kernel(x_cat, x_cont, tables, W_c, b_c, W, b):
    if "nc" not in _cache:
        _cache["nc"] = _build()
    nc = _cache["nc"]
    tab = _prep_tables(tables, W_c, b_c, W, b)
    in_maps = []
    for c in range(N_CORES):
        sl = slice(c * SHARD, (c + 1) * SHARD)
        in_maps.append({
            "tab": tab,
            "idx": _prep_idx(np.asarray(x_cat)[sl], np.asarray(x_cont)[sl]),
        })
    res = run_bass_kernel_spmd(nc, in_maps, core_ids=list(range(N_CORES)))
    outs = []
    for c in range(N_CORES):
        o = res.results[c]["out"].reshape(128, SHARD_PAD // 128, OUT_DIM)
        o = o.transpose(1, 0, 2).reshape(SHARD_PAD, OUT_DIM)[:SHARD]
        outs.append(o)
    return np.ascontiguousarray(np.concatenate(outs, axis=0))

